# revision 9
# baseline (speedup 1.0000x reference)
"""GAT layer kernel for Trainium2, 8 NeuronCores, data-parallel over R=b*s.

Self-contained: takes full inputs, returns full output.

v4 design (per core, RC=6 replicas):
  - Projection on PE in bf16, nt-outer loop: for each 125-node block, all 6
    replicas' h plus the a_src columns are assembled in SBUF and written to
    HBM as one contiguous-row DMA (row n = [h r0..r5 | a_src 24] bf16).
    a_dst scalars stay in SBUF (asad[125, nt, r, 4:8]).
  - Edge phase chunked by dst-tile (125 dsts, dst-sorted slots padded to
    128-slot tiles). Gather descriptors are pre-generated on GpSimd via
    dma_gather(prepare_only=True) and fired with trigger_dma so desc-gen
    overlaps the previous chunk's DMA; 3 gather buffers decouple the
    trigger's buffer-reuse wait from the next chunk's desc-gen.
  - z = a_src[src] (gathered) + a_dst[dst] (PE expand via transposed one-hot)
    p = exp(leaky_relu(z)) written back into the gathered rows' a_src slot;
    msg = hg * p (DVE, bf16); num+den = one PE pass over rhs cols [0:1560]
    (den = trailing 24 p columns); out = sum_h 0.25/den * num + bias.
  - One-hot + transposed one-hot stream from HBM per chunk as one bf16 slab.
"""

import math
import numpy as np
import ml_dtypes

B, S, N, F = 4, 12, 1000, 64
H, C = 4, 64
HC = H * C            # 256
R = B * S             # 48
NCORES = 8
RC = R // NCORES      # 6 replicas per core
NEG_SLOPE = 0.2
DTW = 125             # dst-tile width (8 tiles cover N=1000)
NDT = N // DTW        # 8
AC = RC * H           # 24 active scalar columns
HW = RC * HC          # 1536 h columns per row
ROWU = HW + AC        # 1560 used row width
ROWW = 1664           # h_hbm row width in bf16 (13*128; gather elem size)
OHW = DTW + 128       # merged one-hot slab width per tile (oh | ohT)

_CACHE = {}


# --------------------------------------------------------------------------
# host-side index preprocessing
# --------------------------------------------------------------------------
def _prep_edges(edge_index):
    src0 = np.asarray(edge_index[0], dtype=np.int64)
    dst0 = np.asarray(edge_index[1], dtype=np.int64)
    keep = src0 != dst0                      # PyG remove_self_loops + NEG_INF mask
    s_all = np.concatenate([src0[keep], np.arange(N, dtype=np.int64)])
    d_all = np.concatenate([dst0[keep], np.arange(N, dtype=np.int64)])
    order = np.argsort(d_all, kind="stable")
    s_all, d_all = s_all[order], d_all[order]

    # per dst-tile slot lists, each padded to a multiple of 128
    chunks = []
    for dt in range(NDT):
        lo, hi = dt * DTW, (dt + 1) * DTW
        m = (d_all >= lo) & (d_all < hi)
        ss, dd = s_all[m], d_all[m]
        cnt = len(ss)
        ntile = max(1, math.ceil(cnt / 128))
        pad = ntile * 128 - cnt
        ss = np.concatenate([ss, np.full(pad, 1000, np.int64)])   # pad -> row 1000
        dd = np.concatenate([dd, np.full(pad, lo, np.int64)])
        real = np.concatenate([np.ones(cnt, bool), np.zeros(pad, bool)])
        # one-hot [p, t, dlocal] and transposed [t, dlocal, p]
        oh = np.zeros((128, ntile, DTW), np.float32)
        for j in range(ntile * 128):
            if real[j]:
                oh[j % 128, j // 128, dd[j] - lo] = 1.0
        chunks.append(dict(ntile=ntile, src=ss, oh=oh,
                           ohT=np.ascontiguousarray(oh.transpose(2, 1, 0))))

    maxt = max(c["ntile"] for c in chunks)
    T = sum(c["ntile"] for c in chunks)
    # index tensor: per chunk, slots wrapped [16, slots/16], replicated to 128
    ihw = np.zeros((128, T * 8), np.int16)   # 128 slots = 8 idx columns
    ohc_all = np.zeros((128, T, OHW), ml_dtypes.bfloat16)
    t0 = 0
    for c in chunks:
        nt_, ss = c["ntile"], c["src"]
        ni = nt_ * 128
        a = np.zeros((16, ni // 16), np.int16)
        a[np.arange(ni) % 16, np.arange(ni) // 16] = ss.astype(np.int16)
        ihw[:, t0 * 8:(t0 + nt_) * 8] = np.tile(a, (8, 1))
        ohc_all[:, t0:t0 + nt_, 0:DTW] = c["oh"]
        ohc_all[:DTW, t0:t0 + nt_, DTW:OHW] = c["ohT"].reshape(DTW, nt_, 128)
        t0 += nt_
    return {
        "T": T, "maxt": maxt, "ntiles": [c["ntile"] for c in chunks],
        "ohc": np.ascontiguousarray(ohc_all.reshape(128, T * OHW)),
        "ih": ihw,
    }


def _prep_weights(W, att_src, att_dst):
    W = np.asarray(W, np.float32)
    Ws = np.zeros((F, H), np.float32)
    Wd = np.zeros((F, H), np.float32)
    for h in range(H):
        Ws[:, h] = W[:, h * C:(h + 1) * C] @ np.asarray(att_src, np.float32)[h]
        Wd[:, h] = W[:, h * C:(h + 1) * C] @ np.asarray(att_dst, np.float32)[h]
    # c-major head interleave: device col c*4+h = W col h*64+c
    Wc = np.empty_like(W)
    for h in range(H):
        Wc[:, np.arange(C) * H + h] = W[:, h * C:(h + 1) * C]
    return np.concatenate([Wc, Ws, Wd], axis=1).astype(ml_dtypes.bfloat16)


def _make_in_maps(x, W, att_src, att_dst, bias, ed):
    waug = _prep_weights(W, att_src, att_dst)
    bias_slab = np.tile(np.asarray(bias, np.float32)[None, :],
                        (128, RC)).reshape(128, RC * F)
    xr = np.ascontiguousarray(np.asarray(x, np.float32)).reshape(R, N, F)
    in_maps = []
    for cidx in range(NCORES):
        xc = xr[cidx * RC:(cidx + 1) * RC]
        xT = np.ascontiguousarray(
            xc.transpose(2, 0, 1).reshape(F, RC * N)).astype(ml_dtypes.bfloat16)
        in_maps.append({
            "xT": xT, "w_aug": waug, "ohc": ed["ohc"],
            "ih": ed["ih"], "bias_slab": bias_slab,
        })
    return in_maps


# --------------------------------------------------------------------------
# device program
# --------------------------------------------------------------------------
def _build_program(ed):
    import concourse.bass as bass
    import concourse.mybir as mybir
    import concourse.tile as tile
    from concourse import bacc

    T, maxt = ed["T"], ed["maxt"]
    ntiles = ed["ntiles"]
    f32 = mybir.dt.float32
    bf16 = mybir.dt.bfloat16
    i16 = mybir.dt.int16
    Alu = mybir.AluOpType
    Act = mybir.ActivationFunctionType

    nc = bacc.Bacc("TRN2", target_bir_lowering=False, debug=False,
                   enable_asserts=False, num_devices=NCORES)

    xT_d = nc.dram_tensor("xT", [F, RC * N], bf16, kind="ExternalInput").ap()
    waug_d = nc.dram_tensor("w_aug", [F, 264], bf16, kind="ExternalInput").ap()
    ohc_d = nc.dram_tensor("ohc", [128, T * OHW], bf16, kind="ExternalInput").ap()
    ih_d = nc.dram_tensor("ih", [128, T * 8], i16, kind="ExternalInput").ap()
    bias_d = nc.dram_tensor("bias_slab", [128, RC * F], f32, kind="ExternalInput").ap()
    out_d = nc.dram_tensor("out", [RC, N, F], f32, kind="ExternalOutput").ap()

    # chunk start tile offsets
    t0s = [0]
    for nt_ in ntiles:
        t0s.append(t0s[-1] + nt_)

    with tile.TileContext(nc) as tc:
        with (
            tc.tile_pool(name="const", bufs=1) as constp,
            tc.tile_pool(name="dram", bufs=1, space="DRAM") as dramp,
            tc.tile_pool(name="ohst", bufs=2) as ohstp,
            tc.tile_pool(name="edge", bufs=2) as edgep,
            tc.tile_pool(name="fin", bufs=2) as finp,
            tc.tile_pool(name="epsum", bufs=1, space="PSUM") as epsum,
            tc.tile_pool(name="npsum", bufs=2, space="PSUM") as npsum,
            tc.tile_pool(name="ppsum", bufs=3, space="PSUM") as ppsum,
        ):
            h_hbm = dramp.tile([N + 1, ROWW], bf16)

            # ---- constants (scalar HWDGE queue; sync queue carries x/h/out) --
            waug = constp.tile([F, 264], bf16)
            nc.scalar.dma_start(waug[:], waug_d)
            ih = constp.tile([128, T * 8], i16)
            nc.scalar.dma_start(ih[:], ih_d)
            bias_sl = constp.tile([128, RC, F], f32)
            nc.scalar.dma_start(bias_sl[:], bias_d.rearrange("p (r f) -> p r f", f=F))
            xt = constp.tile([F, RC * N], bf16)
            nc.sync.dma_start(xt[:], xT_d)

            # three gather destination buffers (ping-pong-pung)
            hgs = [constp.tile([128, maxt, ROWW], bf16, name=f"hg{i}")
                   for i in range(3)]
            dma_sems = [nc.alloc_semaphore(f"swdge_dma{i}")
                        for i in range(NDT)]

            # pad row 1000: h-part zeros, as-part -1000 => p == 0 for pad
            # slots (carved out of hg1, which chunk 1 overwrites later)
            padrow = hgs[1][0:1, 0, :]
            nc.vector.memset(padrow, 0.0)
            nc.vector.memset(padrow[:, HW:ROWU], -1000.0)
            nc.sync.dma_start(h_hbm[N:N + 1, :], padrow)

            # ---- phase A: projection; fills h_hbm + asad ----
            # h staging slab carved out of hg2 (first overwritten by chunk 2)
            hsl = hgs[2][:, :, :].rearrange("p t e -> p (t e)")[
                0:DTW, 0:NDT * ROWU].rearrange("p (a e) -> p a e", e=ROWU)
            asad = constp.tile([DTW, NDT, RC, 8], bf16)
            for nt in range(NDT):
                n0 = nt * DTW
                for r in range(RC):
                    ps = ppsum.tile([DTW, 264], f32, tag="proj")
                    nc.tensor.matmul(out=ps[:], lhsT=xt[:, r * N + n0:
                                                        r * N + n0 + DTW],
                                     rhs=waug[:], start=True, stop=True)
                    nc.vector.tensor_copy(out=hsl[:, nt, r * HC:(r + 1) * HC],
                                          in_=ps[:, 0:HC])
                    nc.scalar.copy(out=asad[:, nt, r, :], in_=ps[:, HC:HC + 8])
                # a_src columns for this node block, then one contiguous write
                nc.vector.tensor_copy(out=hsl[:, nt, HW:ROWU].rearrange(
                    "p (r f) -> p r f", f=4), in_=asad[:, nt, :, 0:4])
                nc.sync.dma_start(h_hbm[n0:n0 + DTW, 0:ROWU], hsl[:, nt, :])

            def issue_prep(dt):
                nt_ = ntiles[dt]
                ni = nt_ * 128
                t0 = t0s[dt]
                nc.gpsimd.dma_gather(
                    out_ap=hgs[dt % 3][:, 0:nt_, :], in_ap=h_hbm[:],
                    idxs_ap=ih[:, t0 * 8:(t0 + nt_) * 8],
                    num_idxs=ni, num_idxs_reg=ni, elem_size=ROWW,
                    single_packet=False, prepare_only=True, sem=dma_sems[dt])

            # prep 0 issued after the h_hbm writes so the deferred data dep
            # (resolved at trigger time) covers them
            issue_prep(0)

            # ---- per dst-tile chunks ----
            for dt in range(NDT):
                nt_ = ntiles[dt]
                t0 = t0s[dt]
                hg = hgs[dt % 3]
                nc.gpsimd.trigger_dma(count=None)    # fire chunk dt's gather
                if dt + 1 < NDT:
                    issue_prep(dt + 1)               # desc-gen under DMA dt

                # one-hot slab for this chunk ([.., 0:125]=oh, [.., 125:253]=ohT)
                ohc = ohstp.tile([128, maxt, OHW], bf16, tag="ohc")
                nc.scalar.dma_start(
                    ohc[:, 0:nt_, :],
                    ohc_d[:, t0 * OHW:(t0 + nt_) * OHW].rearrange(
                        "p (t d) -> p t d", d=OHW))

                # ad expand: [128(e), nt_, 24] psum via transposed one-hot
                eps = epsum.tile([128, maxt, AC], f32, tag="eps")
                for t in range(nt_):
                    nc.tensor.matmul(out=eps[:, t, :],
                                     lhsT=ohc[0:DTW, t, DTW:OHW],
                                     rhs=asad[:, dt, :, 4:8], start=True,
                                     stop=True)
                # z = as + ad ; leaky relu ; p = exp(z) into the as slot (bf16)
                # (manual gate: Tile doesn't thread the SWDGE completion sem
                #  to SBUF consumers of a prepare_only gather; all other hg
                #  accesses chain transitively behind this vector wait)
                nc.vector.wait_ge(dma_sems[dt], 16)
                z = edgep.tile([128, maxt, AC], f32, tag="z")
                nc.vector.tensor_tensor(
                    out=z[:, 0:nt_, :],
                    in0=hg[:, 0:nt_, HW:ROWU],
                    in1=eps[:, 0:nt_, :], op=Alu.add)
                nc.vector.scalar_tensor_tensor(
                    out=z[:, 0:nt_, :], in0=z[:, 0:nt_, :], scalar=NEG_SLOPE,
                    in1=z[:, 0:nt_, :], op0=Alu.mult, op1=Alu.max)
                nc.scalar.activation(out=hg[:, 0:nt_, HW:ROWU],
                                     in_=z[:, 0:nt_, :], func=Act.Exp)

                # msg = hg * p (per replica, heads broadcast over c-major;
                # replicas 3-5 first so the np1 matmuls can start sooner)
                for r in (3, 4, 5, 0, 1, 2):
                    hgr = hg[:, 0:nt_, r * HC:(r + 1) * HC].rearrange(
                        "p t (c h) -> p t c h", h=H)
                    pb = hg[:, 0:nt_, HW + 4 * r:HW + 4 * r + 4].rearrange(
                        "p t (o h) -> p t o h", o=1).to_broadcast(
                        [128, nt_, C, H])
                    nc.vector.tensor_tensor(out=hgr, in0=hgr, in1=pb,
                                            op=Alu.mult)

                # half 1 first (its rhs tail carries the den columns)
                np1 = npsum.tile([DTW, 792], f32, tag="np")
                for t in range(nt_):
                    nc.tensor.matmul(out=np1[:, 0:512], lhsT=ohc[:, t, 0:DTW],
                                     rhs=hg[:, t, 768:1280],
                                     start=(t == 0), stop=(t == nt_ - 1))
                for t in range(nt_):
                    nc.tensor.matmul(out=np1[:, 512:792], lhsT=ohc[:, t, 0:DTW],
                                     rhs=hg[:, t, 1280:1560],
                                     start=(t == 0), stop=(t == nt_ - 1))
                denrec = edgep.tile([DTW, AC], f32, tag="denrec")
                nc.vector.reciprocal(out=denrec[:], in_=np1[:, 768:792])
                nc.vector.tensor_scalar_mul(denrec[:], denrec[:], 0.25)

                np0 = npsum.tile([DTW, 792], f32, tag="np")
                for t in range(nt_):
                    nc.tensor.matmul(out=np0[:, 0:512], lhsT=ohc[:, t, 0:DTW],
                                     rhs=hg[:, t, 0:512],
                                     start=(t == 0), stop=(t == nt_ - 1))
                for t in range(nt_):
                    nc.tensor.matmul(out=np0[:, 512:768], lhsT=ohc[:, t, 0:DTW],
                                     rhs=hg[:, t, 512:768],
                                     start=(t == 0), stop=(t == nt_ - 1))

                # finalize both halves: num*denrec, head-sum, +bias, DMA out
                for half, nps in ((1, np1), (0, np0)):
                    n4 = nps[:, 0:768].rearrange("d (r c h) -> d r c h",
                                                 h=H, c=C)
                    numn = finp.tile([DTW, RC // 2, C, H], f32, tag="numn")
                    drb = denrec[:, half * 12:half * 12 + 12].rearrange(
                        "d (r o h) -> d r o h", h=H, o=1).to_broadcast(
                        [DTW, RC // 2, C, H])
                    nc.vector.tensor_tensor(out=numn[:], in0=n4, in1=drb,
                                            op=Alu.mult)
                    t1 = finp.tile([DTW, RC // 2, C], f32, tag="t1")
                    t2 = finp.tile([DTW, RC // 2, C], f32, tag="t2")
                    ob = finp.tile([DTW, RC // 2, C], f32, tag="ob")
                    nc.vector.tensor_tensor(out=t1[:], in0=numn[:, :, :, 0],
                                            in1=numn[:, :, :, 1], op=Alu.add)
                    nc.vector.tensor_tensor(out=t2[:], in0=numn[:, :, :, 2],
                                            in1=numn[:, :, :, 3], op=Alu.add)
                    nc.vector.tensor_tensor(out=t1[:], in0=t1[:], in1=t2[:],
                                            op=Alu.add)
                    nc.vector.tensor_tensor(
                        out=ob[:], in0=t1[:],
                        in1=bias_sl[0:DTW, half * 3:half * 3 + 3, :],
                        op=Alu.add)
                    nc.sync.dma_start(
                        out_d[half * 3:half * 3 + 3,
                              dt * DTW:(dt + 1) * DTW, :].rearrange(
                            "r d f -> d r f"), ob[:])

    nc.compile()
    return nc


# --------------------------------------------------------------------------
# public entry point
# --------------------------------------------------------------------------
def kernel(x, edge_index, W, att_src, att_dst, bias):
    key = hash(np.asarray(edge_index).tobytes())
    if key not in _CACHE:
        ed = _prep_edges(edge_index)
        _CACHE[key] = (_build_program(ed), ed)
    nc, ed = _CACHE[key]

    in_maps = _make_in_maps(x, W, att_src, att_dst, bias, ed)
    from concourse import bass_utils
    res = bass_utils.run_bass_kernel_spmd(nc, in_maps, core_ids=list(range(NCORES)))
    outs = [res.results[c]["out"] for c in range(NCORES)]
    out = np.concatenate(outs, axis=0).reshape(B, S, N, F).astype(np.float32)
    return out
